# revision 18
# baseline (speedup 1.0000x reference)
"""Compositional attention kernel for Trainium2, 8-core SPMD.

Sharding: core c handles batch b = c // 4 and search-heads {2*(c%4), 2*(c%4)+1}
end-to-end (tensor-parallel over the S=8 search heads x data-parallel over
B=2).  Each core computes a partial y = out_heads @ w_out[head_rows] for its
batch; the host sums the 4 partials per batch.

All matmuls run in fp16 (1 cycle/row on the PE) with fp32 PSUM accumulation;
softmax denominators, stage-2 gating and all reductions stay fp32.  Host-side
prep: x is pre-transposed, the DH**-0.5 scales are folded into wq/wq_r, and
wk_ret is pre-transposed; everything is pre-cast to fp16.

Per-core pipeline (2 heads, n=2048, dim=512, DH=64, R=2):
  sqT/skT/rqT = W^T x^T   [128=(2h x 64d), n]    (fp16 matmuls)
  rv          = x wv      [n, (r d)=128] fp16    (via rvT + PE transpose)
  per head h:
    ST[j,i] = skT_h^T sqT_h        (scores, transposed layout)
    ET      = exp(ST)              (ACT, fp16; |scores| < 9, no max-sub)
    uT[rd,i]= rv^T ET              (unnormalized retrieved^T)
    Z[i]    = 1^T ET               (DVE 4->1 tile tree + ones-matmuls)
    u       = uT^T (PE transpose); rqW = rq wk_ret^T
    sim~_r  = sum_d u_r * rqW ; attn0 = sigmoid((sim~0-sim~1)/Z)
    u_c     = (attn0/Z) u_0 + ((1-attn0)/Z) u_1
  y = sum_h u_c_h^T @ w_out_h      (K=64 PSUM-accumulated matmuls)
"""

import os  # noqa: F401
import sys

import numpy as np

for _p in ("/opt/trn_rl_repo", "/root/.axon_site/_ro/trn_rl_repo"):
    if _p not in sys.path:
        sys.path.append(_p)

import concourse.bass as bass  # noqa: F401
import concourse.mybir as mybir
import concourse.tile as tile
from concourse import bacc
from concourse.bass_utils import run_bass_kernel_spmd
from concourse.masks import make_identity

S, R, DH = 8, 2, 64
B, N, DIM = 2, 2048, 512
P = 128
NT = N // P  # 16 n-tiles
KC = DIM // P  # 4 contraction chunks of x
NCORES = 8
HPC = 2  # heads per core

F32 = mybir.dt.float32
F16 = mybir.dt.float16
AF = mybir.ActivationFunctionType
ALU = mybir.AluOpType


def _emit(tc, xt, wq, wk, wv, wqr, wkt, wo, y, zscr):
    from contextlib import ExitStack

    nc = tc.nc
    with ExitStack() as ctx:
        cpool = ctx.enter_context(tc.tile_pool(name="const", bufs=1))
        xs = ctx.enter_context(tc.tile_pool(name="xs", bufs=2))
        sb = ctx.enter_context(tc.tile_pool(name="sb", bufs=1))
        hd = ctx.enter_context(tc.tile_pool(name="hd", bufs=2))
        ps = ctx.enter_context(tc.tile_pool(name="ps", bufs=2, space="PSUM"))
        zps = ctx.enter_context(tc.tile_pool(name="zps", bufs=2, space="PSUM"))

        # ---- constants ----
        ident = cpool.tile([P, P], F16, name="ident")
        make_identity(nc, ident)
        ones16 = cpool.tile([P, 1], F16, name="ones16")
        nc.gpsimd.memset(ones16, 1.0)

        wq_sb = cpool.tile([P, KC, P], F16, name="wq_sb")
        nc.sync.dma_start(wq_sb, wq.rearrange("(kc p) m -> p kc m", p=P))
        wk_sb = cpool.tile([P, KC, P], F16, name="wk_sb")
        nc.sync.dma_start(wk_sb, wk.rearrange("(kc p) m -> p kc m", p=P))
        wqr_sb = cpool.tile([P, KC, P], F16, name="wqr_sb")
        nc.sync.dma_start(wqr_sb, wqr.rearrange("(kc p) m -> p kc m", p=P))
        wv_sb = cpool.tile([P, KC, P], F16, name="wv_sb")
        nc.sync.dma_start(wv_sb, wv.rearrange("(kc p) m -> p kc m", p=P))
        # wk_ret^T replicated on both 64-partition halves (head 1's rqW
        # matmul reads its lhsT at base partition 64; rhs must match).
        wkt_sb = cpool.tile([2 * DH, DH], F16, name="wkt_sb")
        nc.sync.dma_start(wkt_sb[0:DH, :], wkt[:, :])
        nc.sync.dma_start(wkt_sb[DH : 2 * DH, :], wkt[:, :])
        wo0_sb = cpool.tile([DH, DIM], F16, name="wo0_sb")
        nc.sync.dma_start(wo0_sb, wo[0:DH, :])
        wo1_sb = cpool.tile([DH, DIM], F16, name="wo1_sb")
        nc.sync.dma_start(wo1_sb, wo[DH : 2 * DH, :])

        # ---- x^T comes pre-transposed (and pre-cast) from the host ----
        xT = sb.tile([P, KC, N], F16, name="xT")
        for kc in range(KC):
            nc.sync.dma_start(xT[:, kc, :], xt[kc * P : (kc + 1) * P, :])

        # ---- projections: sqT/skT/rqT [128=(2 heads x 64), n] fp16 ----
        sqT = sb.tile([P, N], F16, name="sqT")
        skT = sb.tile([P, N], F16, name="skT")
        rqT = sb.tile([P, N], F16, name="rqT")
        for wsb, dst in ((wq_sb, sqT), (wk_sb, skT), (wqr_sb, rqT)):
            for ic in range(4):
                pp = ps.tile([P, 512], F32, tag="ps1", name="pp")
                for kc in range(KC):
                    nc.tensor.matmul(
                        pp,
                        wsb[:, kc, :],
                        xT[:, kc, ic * 512 : (ic + 1) * 512],
                        start=(kc == 0),
                        stop=(kc == KC - 1),
                    )
                nc.any.tensor_copy(out=dst[:, ic * 512 : (ic + 1) * 512], in_=pp)

        # ---- rv: [n, (r d)=128] fp16, via rvT then PE transpose ----
        rv16 = sb.tile([P, NT, P], F16, name="rv16")
        for ic in range(4):
            pv = ps.tile([P, 512], F32, tag="ps1", name="pv")
            for kc in range(KC):
                nc.tensor.matmul(
                    pv,
                    wv_sb[:, kc, :],
                    xT[:, kc, ic * 512 : (ic + 1) * 512],
                    start=(kc == 0),
                    stop=(kc == KC - 1),
                )
            rvT_c = hd.tile([P, 512], F16, tag="rvT", name="rvT_c", bufs=1)
            nc.any.tensor_copy(out=rvT_c, in_=pv)
            for t in range(4):
                jt = ic * 4 + t
                pt = ps.tile([P, P], F16, tag="ps1", name="pt")
                nc.tensor.transpose(pt, rvT_c[:, t * P : (t + 1) * P], ident)
                nc.any.tensor_copy(out=rv16[:, jt, :], in_=pt)

        # ---- per-head attention ----
        ET = sb.tile([P, NT, N], F16, name="ET")  # [j_local, j_tile, i]
        uTfin = []
        for h in range(HPC):
            hs = slice(h * DH, (h + 1) * DH)
            # scores (transposed layout [j, i]) + exp
            for jt in range(NT):
                for icc in range(2):
                    st = ps.tile([P, 1024], F32, tag="st", name="st")
                    for half in range(2):
                        i0 = icc * 1024 + half * 512
                        nc.tensor.matmul(
                            st[:, half * 512 : (half + 1) * 512],
                            skT[hs, jt * P : (jt + 1) * P],
                            sqT[hs, i0 : i0 + 512],
                            start=True,
                            stop=True,
                        )
                    nc.scalar.activation(
                        ET[:, jt, icc * 1024 : (icc + 1) * 1024], st, AF.Exp
                    )
            # AV in i-chunk pairs, j-outer: consecutive matmuls share the
            # stationary rv tile.
            uT16 = hd.tile([P, N], F16, tag="uT16", name=f"uT16_{h}", bufs=1)
            for icp in range(2):
                pra = ps.tile([P, 512], F32, tag="ps1", name="pra")
                prb = ps.tile([P, 512], F32, tag="ps1", name="prb")
                i0 = icp * 1024
                for jt in range(NT):
                    nc.tensor.matmul(
                        pra,
                        rv16[:, jt, :],
                        ET[:, jt, i0 : i0 + 512],
                        start=(jt == 0),
                        stop=(jt == NT - 1),
                        skip_group_check=True,
                    )
                    nc.tensor.matmul(
                        prb,
                        rv16[:, jt, :],
                        ET[:, jt, i0 + 512 : i0 + 1024],
                        start=(jt == 0),
                        stop=(jt == NT - 1),
                        skip_group_check=True,
                    )
                nc.any.tensor_copy(out=uT16[:, i0 : i0 + 512], in_=pra)
                nc.any.tensor_copy(out=uT16[:, i0 + 512 : i0 + 1024], in_=prb)
            # Z: collapse the 16 j-tiles of exp(S) 4->1 on DVE (in-place,
            # fp16 is safe: Z_max ~1.5e4 << fp16 max), then 4 ones-matmuls
            # per i-chunk reduce the remaining partition dim on PE.
            for g in range(4):
                b0 = 4 * g
                nc.vector.tensor_tensor(ET[:, b0], ET[:, b0], ET[:, b0 + 1], ALU.add)
                nc.vector.tensor_tensor(
                    ET[:, b0 + 2], ET[:, b0 + 2], ET[:, b0 + 3], ALU.add
                )
                nc.vector.tensor_tensor(ET[:, b0], ET[:, b0], ET[:, b0 + 2], ALU.add)
            for ic in range(4):
                pz = zps.tile([1, 512], F32, tag="z", name="pz")
                for g in range(4):
                    nc.tensor.matmul(
                        pz,
                        ones16,
                        ET[:, 4 * g, ic * 512 : (ic + 1) * 512],
                        start=(g == 0),
                        stop=(g == 3),
                    )
                zrow_c = hd.tile([1, 512], F32, tag="zrow", name="zrow_c")
                nc.vector.tensor_copy(out=zrow_c, in_=pz)
                nc.sync.dma_start(
                    zscr[h : h + 1, ic * 512 : (ic + 1) * 512], zrow_c[:, :]
                )
            # Z gather to [i_local, i_tile] layout + reciprocal
            zcol = hd.tile([P, NT], F32, tag="zcol", name="zcol")
            nc.sync.dma_start(zcol, zscr.rearrange("b (it p) -> b p it", p=P)[h])
            zinv = hd.tile([P, NT], F32, tag="zinv", name="zinv")
            nc.vector.reciprocal(zinv, zcol)
            # u natural layout [i_local, (i_tile, r, d)]
            un = hd.tile([P, NT, R, DH], F32, tag="un", name=f"un{h}", bufs=1)
            for it in range(NT):
                pt2 = ps.tile([P, P], F16, tag="ps1", name="pt2")
                nc.tensor.transpose(pt2, uT16[:, it * P : (it + 1) * P], ident)
                nc.any.tensor_copy(out=un[:, it], in_=pt2)
            # rqW = rq @ wk_ret^T, natural layout [i_local, (i_tile, d)]
            rqw = hd.tile([P, NT, DH], F32, tag="rqw", name="rqw", bufs=1)
            for it in range(NT):
                pq = ps.tile([P, DH], F32, tag="ps1", name="pq")
                nc.tensor.matmul(
                    pq,
                    rqT[hs, it * P : (it + 1) * P],
                    wkt_sb[hs, :],
                    start=True,
                    stop=True,
                )
                nc.any.tensor_copy(out=rqw[:, it], in_=pq)
            # stage 2: softmax over R=2 via sigmoid, fold 1/Z in at the end
            prod = hd.tile([P, NT, R, DH], F32, tag="prod", name="prod", bufs=1)
            nc.vector.tensor_tensor(
                prod, un, rqw[:, :, None, :].to_broadcast((P, NT, R, DH)), ALU.mult
            )
            sims = hd.tile([P, NT, R], F32, tag="sims", name="sims")
            nc.vector.tensor_reduce(sims, prod, axis=mybir.AxisListType.X, op=ALU.add)
            gd = hd.tile([P, NT], F32, tag="gd", name="gd")
            nc.vector.tensor_tensor(gd, sims[:, :, 0], sims[:, :, 1], ALU.subtract)
            nc.vector.tensor_tensor(gd, gd, zinv, ALU.mult)
            attn = hd.tile([P, NT], F32, tag="attn", name="attn")
            nc.scalar.activation(attn, gd, AF.Sigmoid)
            aa = hd.tile([P, NT], F32, tag="aa", name="aa")
            nc.vector.tensor_tensor(aa, attn, zinv, ALU.mult)
            u0 = un[:, :, 0, :]
            u1 = un[:, :, 1, :]
            nc.vector.tensor_tensor(u0, u0, u1, ALU.subtract)
            nc.vector.tensor_tensor(
                u1, u1, zinv[:, :, None].to_broadcast((P, NT, DH)), ALU.mult
            )
            nc.vector.tensor_tensor(
                u0, u0, aa[:, :, None].to_broadcast((P, NT, DH)), ALU.mult
            )
            uc16 = hd.tile([P, NT, DH], F16, tag="uc16", name=f"uc16_{h}")
            nc.vector.tensor_tensor(uc16, u0, u1, ALU.add)
            # transpose combined head output back to [d, i] for the out-proj
            uf = hd.tile([DH, N], F16, tag="uf", name=f"uf{h}")
            for it in range(NT):
                pf = ps.tile([DH, P], F16, tag="ps1", name="pf")
                nc.tensor.transpose(pf, uc16[:, it], ident)
                nc.any.tensor_copy(out=uf[:, it * P : (it + 1) * P], in_=pf)
            uTfin.append(uf)

        # ---- output projection: y_partial = sum_h u_c_h @ w_out_h ----
        for it in range(NT):
            py = ps.tile([P, DIM], F32, tag="ps1", name="py")
            nc.tensor.matmul(
                py,
                uTfin[0][:, it * P : (it + 1) * P],
                wo0_sb,
                start=True,
                stop=False,
            )
            nc.tensor.matmul(
                py,
                uTfin[1][:, it * P : (it + 1) * P],
                wo1_sb,
                start=False,
                stop=True,
            )
            ysb = xs.tile([P, DIM], F32, tag="ysb", name="ysb")
            nc.any.tensor_copy(out=ysb, in_=py)
            nc.sync.dma_start(y[it * P : (it + 1) * P, :], ysb)


def build_program():
    nc = bacc.Bacc(None, target_bir_lowering=False)
    xt = nc.declare_dram_parameter("xt", [DIM, N], F16, isOutput=False)
    wq = nc.declare_dram_parameter("wq", [DIM, P], F16, isOutput=False)
    wk = nc.declare_dram_parameter("wk", [DIM, P], F16, isOutput=False)
    wv = nc.declare_dram_parameter("wv", [DIM, P], F16, isOutput=False)
    wqr = nc.declare_dram_parameter("wqr", [DIM, P], F16, isOutput=False)
    wkt = nc.declare_dram_parameter("wkt", [DH, DH], F16, isOutput=False)
    wo = nc.declare_dram_parameter("wo", [P, DIM], F16, isOutput=False)
    y = nc.declare_dram_parameter("y", [N, DIM], F32, isOutput=True)
    zscr = nc.dram_tensor("zscr", [HPC, N], F32)

    with tile.TileContext(nc) as tc:
        _emit(tc, xt, wq, wk, wv, wqr, wkt, wo, y, zscr)
    nc.compile()
    return nc


_NC_CACHE = None


def _get_program():
    global _NC_CACHE
    if _NC_CACHE is None:
        _NC_CACHE = build_program()
    return _NC_CACHE


def make_in_maps(inputs):
    x = np.asarray(inputs["x"], dtype=np.float32)
    wq_s = np.asarray(inputs["wq_s"], dtype=np.float32)
    wk_s = np.asarray(inputs["wk_s"], dtype=np.float32)
    wv_r = np.asarray(inputs["wv_r"], dtype=np.float32)
    wq_r = np.asarray(inputs["wq_r"], dtype=np.float32)
    wk_ret = np.asarray(inputs["wk_ret"], dtype=np.float32)
    w_out = np.asarray(inputs["w_out"], dtype=np.float32)
    scale = np.float32(DH**-0.5)

    f16 = np.float16
    in_maps = []
    for c in range(NCORES):
        b, hp = divmod(c, NCORES // B)
        cols = slice(hp * P, (hp + 1) * P)
        in_maps.append(
            {
                "xt": np.ascontiguousarray(x[b].T).astype(f16),
                "wq": (np.ascontiguousarray(wq_s[:, cols]) * scale).astype(f16),
                "wk": np.ascontiguousarray(wk_s[:, cols]).astype(f16),
                "wv": wv_r.astype(f16),
                "wqr": (np.ascontiguousarray(wq_r[:, cols]) * scale).astype(f16),
                "wkt": np.ascontiguousarray(wk_ret.T).astype(f16),
                "wo": np.ascontiguousarray(w_out[hp * P : (hp + 1) * P, :]).astype(f16),
            }
        )
    return in_maps


def run(inputs, trace=False, **kw):
    res = run_bass_kernel_spmd(
        _get_program(), make_in_maps(inputs), list(range(NCORES)), trace=trace, **kw
    )
    out = np.zeros((B, N, DIM), np.float32)
    for c in range(NCORES):
        out[c // (NCORES // B)] += np.asarray(res.results[c]["y"], np.float32)
    return out, res


def kernel(**inputs):
    out, _ = run(inputs)
    return out
